# revision 1
# baseline (speedup 1.0000x reference)
"""AVLnet baseline model Bass kernel for 8x TRN2 NeuronCores.

Contract: kernel(**inputs) takes the FULL (unsharded) numpy inputs as produced
by setup_inputs() and returns the full (3, 32, 4096) float32 output.

Sharding: pure data parallel over the batch dim - 4 samples per core, weights
replicated. One Bass program is compiled and run SPMD on cores 0-7 via
concourse.bass_utils.run_bass_kernel_spmd; per-core outputs are concatenated
on the host.
"""

import sys

for _p in ("/opt/trn_rl_repo", "/root/.axon_site/_ro/trn_rl_repo"):
    if _p not in sys.path:
        sys.path.append(_p)

from contextlib import ExitStack

import numpy as np

import concourse.bass as bass
import concourse.mybir as mybir
import concourse.tile as tile
from concourse import bacc
from concourse.masks import make_identity

F32 = mybir.dt.float32
F32R = mybir.dt.float32r
F16 = mybir.dt.float16
I32 = mybir.dt.int32
AF = mybir.ActivationFunctionType
ALU = mybir.AluOpType

NEG = -3.0e38  # effectively -inf for max-pool padding
NS = 4  # samples per core


def declare_io(nc, debug=False):
    d = {}

    def inp(name, shape, dt):
        d[name] = nc.dram_tensor(name, list(shape), dt, kind="ExternalInput")

    # per-core data
    inp("aT", (40, NS * 2048), F32R)          # audio, (freq, sample*time)
    inp("tT", (300, NS * 30), F32R)           # text, (emb, sample*word)
    inp("vT", (4096, NS * 16), F32R)          # video, (dim, sample*clip)
    inp("nf", (NS, 1), I32)                   # audio_STFT_nframes
    # conv weights (replicated); layout (coutp*cinp, cin128, tap*cout128)
    inp("w1T", (40, 128), F32R)
    inp("b1", (128, 1), F32)
    inp("w2", (2, 128, 11 * 128), F32R)
    inp("b2", (128, 2), F32)
    inp("w3", (4 * 2, 128, 17 * 128), F32R)
    inp("b3", (128, 4), F32)
    inp("w4", (4 * 4, 128, 17 * 128), F32R)
    inp("b4", (128, 4), F32)
    inp("w5", (8 * 4, 128, 17 * 128), F32R)
    inp("b5", (128, 8), F32)
    # text branch
    inp("tpT", (300, 4096), F32R)
    inp("tpb", (128, 32), F32)
    # GEU weights, transposed to (in, out)
    # GEU weights repacked host-side as (og, p, k*1024): per-partition
    # contiguous runs for max DMA descriptor size.
    for nm in ("gutf", "gutc", "guvf", "guvc"):
        inp(nm + "T", (4, 128, 32 * 1024), F16)
        inp(nm + "b", (1, 4096), F16)
    for nm in ("guaf", "guac"):
        inp(nm + "T", (1, 128, 8 * 1024), F32R)
        inp(nm + "b", (1, 1024), F32R)
    inp("projT", (4, 128, 8 * 1024), F32R)
    inp("projb", (1, 4096), F32R)

    d["out"] = nc.dram_tensor("out", [3, NS, 4096], F32, kind="ExternalOutput")
    if debug:
        d["dbg_x2"] = nc.dram_tensor("dbg_x2", [128, NS * 2058], F32, kind="ExternalOutput")
        d["dbg_x3"] = nc.dram_tensor("dbg_x3", [2 * 128, NS * 1040], F32, kind="ExternalOutput")
        d["dbg_x5"] = nc.dram_tensor("dbg_x5", [4 * 128, NS * 272], F32, kind="ExternalOutput")
        d["dbg_a"] = nc.dram_tensor("dbg_a", [8 * 128, NS * 128], F32, kind="ExternalOutput")
        d["dbg_pool"] = nc.dram_tensor("dbg_pool", [128, 8 * NS], F32, kind="ExternalOutput")
        d["dbg_t"] = nc.dram_tensor("dbg_t", [128, 32 * NS], F32, kind="ExternalOutput")
        d["dbg_v"] = nc.dram_tensor("dbg_v", [128, 32 * NS], F32, kind="ExternalOutput")
        d["dbg_gx1"] = nc.dram_tensor("dbg_gx1", [NS, 4096], F32, kind="ExternalOutput")
        d["dbg_gx2"] = nc.dram_tensor("dbg_gx2", [NS, 4096], F32, kind="ExternalOutput")
    return d


def emit(ctx: ExitStack, tc: tile.TileContext, d, debug=False):
    nc = tc.nc

    # ---------------- pools ----------------
    consts = ctx.enter_context(tc.tile_pool(name="consts", bufs=1))
    acts = ctx.enter_context(tc.tile_pool(name="acts", bufs=1))
    wstream = ctx.enter_context(tc.tile_pool(name="wstream", bufs=3))
    ystream = ctx.enter_context(tc.tile_pool(name="ystream", bufs=2))
    geu_sb = ctx.enter_context(tc.tile_pool(name="geu_sb", bufs=1))
    gstream = ctx.enter_context(tc.tile_pool(name="gstream", bufs=3))
    small = ctx.enter_context(tc.tile_pool(name="small", bufs=2))

    psum_conv = ctx.enter_context(tc.tile_pool(name="psum_conv", bufs=2, space="PSUM"))
    psum_geu = ctx.enter_context(tc.tile_pool(name="psum_geu", bufs=1, space="PSUM"))
    psum_tp = ctx.enter_context(tc.tile_pool(name="psum_tp", bufs=2, space="PSUM"))
    psum_ss = ctx.enter_context(tc.tile_pool(name="psum_ss", bufs=1, space="PSUM"))

    # ---------------- constants ----------------
    ident = consts.tile([NS, NS], F32)
    make_identity(nc, ident[:])
    ones_f = consts.tile([128, 1], F32)
    nc.vector.memset(ones_f[:], 1.0)
    ones_r = consts.tile([128, 1], F32R)
    nc.vector.tensor_copy(ones_r[:], ones_f[:])
    ones_row_f = consts.tile([1, NS], F32)
    nc.vector.memset(ones_row_f[:], 1.0)
    ones_row_r = consts.tile([1, NS], F32R)
    nc.vector.tensor_copy(ones_row_r[:], ones_row_f[:])
    ones_row_h = consts.tile([1, NS], F16)
    nc.vector.tensor_copy(ones_row_h[:], ones_row_f[:])
    zeros64 = consts.tile([128, 128], F32)
    nc.vector.memset(zeros64[:], 0.0)

    b1t = consts.tile([128, 1], F32); nc.sync.dma_start(b1t[:], d["b1"][:, :])
    b2t = consts.tile([128, 2], F32); nc.sync.dma_start(b2t[:], d["b2"][:, :])
    b3t = consts.tile([128, 4], F32); nc.sync.dma_start(b3t[:], d["b3"][:, :])
    b4t = consts.tile([128, 4], F32); nc.sync.dma_start(b4t[:], d["b4"][:, :])
    b5t = consts.tile([128, 8], F32); nc.sync.dma_start(b5t[:], d["b5"][:, :])
    tpbt = consts.tile([128, 32], F32); nc.sync.dma_start(tpbt[:], d["tpb"][:, :])

    # ---------------- mask for audio masked-mean ----------------
    nfi = small.tile([NS, 1], I32)
    nc.sync.dma_start(nfi[:], d["nf"][:, :])
    nfs = small.tile([NS, 1], I32)
    nc.vector.tensor_scalar(nfs[:], nfi[:], 4, None, ALU.arith_shift_right)
    nfs2 = small.tile([NS, 1], I32)
    nc.vector.tensor_scalar_max(nfs2[:], nfs[:], 1)
    nff = small.tile([NS, 1], F32)
    nc.vector.tensor_copy(nff[:], nfs2[:])
    rnf = small.tile([NS, 1], F32)
    nc.vector.reciprocal(rnf[:], nff[:])
    iot = small.tile([NS, 128], I32)
    nc.gpsimd.iota(iot[:], pattern=[[1, 128]], base=0, channel_multiplier=0)
    iotf = small.tile([NS, 128], F32)
    nc.vector.tensor_copy(iotf[:], iot[:])
    mrow = small.tile([NS, 128], F32)
    nc.vector.tensor_scalar(mrow[:], iotf[:], nff[:], None, ALU.is_lt)
    mrow2 = small.tile([NS, 128], F32)
    nc.vector.tensor_scalar_mul(mrow2[:], mrow[:], rnf[:])
    mbs = []
    for s in range(NS):
        stage = small.tile([1, 128], F32, name=f"mstage{s}", tag="mstage")
        nc.sync.dma_start(stage[:], mrow2[s:s + 1, :])
        mb = consts.tile([128, 128], F32, name=f"mb{s}")
        nc.gpsimd.partition_broadcast(mb[:], stage[:])
        mbs.append(mb)

    # ---------------- activation buffers (slot-shared) ----------------
    # slot1: X2 (conv2 in) -> X4 (conv4 in) -> A (pooled conv5 out)
    # slot2: X3 (conv3 in) -> X5 (conv5 in)
    X2 = acts.tile([128, NS * 2058], F32R, tag="slot1")
    X3 = acts.tile([128, 2 * NS * 1040], F32R, tag="slot2")

    def x3c(c):  # chunk c of conv3 input channels
        return X3[:, c * (NS * 1040):(c + 1) * (NS * 1040)]

    def zero_halos(buf, nchunk, blk, pad):
        v = buf.rearrange("p (c s x) -> p c s x", c=nchunk, s=NS)
        zin = zeros64[:, 0:nchunk * NS * pad].rearrange(
            "p (c s x) -> p c s x", c=nchunk, s=NS)
        nc.vector.tensor_copy(v[:, :, :, 0:pad], zin)
        nc.vector.tensor_copy(v[:, :, :, blk - pad:blk], zin)

    zero_halos(X2[:, :], 1, 2058, 5)
    zero_halos(X3[:, :], 2, 1040, 8)

    # ---------------- generic flipped-operand GEU ----------------
    def transpose_to_chunks(xbuf, D, dst, dst_dt):
        """xbuf [NS, D] f32 -> dst [128, (D//128)*NS] chunks of x.T"""
        for k in range(D // 128):
            tp = psum_tp.tile([128, NS], F32, tag="tpp")
            nc.tensor.transpose(tp[:], xbuf[:, k * 128:(k + 1) * 128], ident[0:NS, 0:NS])
            nc.scalar.copy(dst[:, k * NS:(k + 1) * NS], tp[:])

    class GeuPlan:
        """GatedEmbeddingUnit staged into (dma, mm) item pairs for interleaving."""

        def __init__(self, name, xT_fn, nk, D, fkey, ckey, wdt, out_row=None):
            self.name, self.xT_fn, self.nk, self.D = name, xT_fn, nk, D
            self.fkey, self.ckey, self.wdt, self.out_row = fkey, ckey, wdt, out_row
            self.KI = 4 if wdt == F16 else 2  # k-chunks per weight-stream item
            self.x1 = self.x1T = self.x2 = self.ssb = self.inv = None

        def items(self):
            n_og = self.D // 1024
            for og in range(n_og):
                yield from self.lin_items(og, self.fkey, self.xT_fn, self.nk,
                                          self.lin1_epi)
            yield (None, self.mid)
            for og in range(n_og):
                yield from self.lin_items(og, self.ckey, lambda: self.x1T,
                                          self.D // 128, self.gate_epi)
            if self.out_row is not None:
                yield (None, self.fin)

        def lin_items(self, og, wkey, xT_fn, nk, epi):
            st = {}
            wT_d, bias_d = d[wkey + "T"], d[wkey + "b"]
            KI = self.KI
            n_items = (nk + KI - 1) // KI
            o0 = og * 1024

            def dma_fn(i):
                if i == 0:
                    st["pss"] = [psum_geu.tile([NS, 512], F32, tag=f"gps{j}",
                                               name=f"gps{j}") for j in range(2)]
                    brow = small.tile([1, 1024], self.wdt, tag="brow", name="brow")
                    nc.sync.dma_start(brow[:], bias_d[0:1, o0:o0 + 1024])
                    st["brow"] = brow
                kc = min(KI, nk - i * KI)
                wt = gstream.tile([128, KI * 1024], self.wdt, tag="gw", name="gw")
                nc.sync.dma_start(wt[:, 0:kc * 1024],
                                  wT_d[og, :, i * KI * 1024:(i * KI + kc) * 1024])
                st[i] = wt

            def mm_fn(i):
                wt = st.pop(i)
                pss = st["pss"]
                xT = xT_fn()
                kc = min(KI, nk - i * KI)
                for k in range(kc):
                    kk = i * KI + k
                    for j in range(2):
                        nc.tensor.matmul(pss[j][:], xT[:, kk * NS:(kk + 1) * NS],
                                         wt[:, k * 1024 + j * 512: k * 1024 + (j + 1) * 512],
                                         start=(kk == 0), stop=False)
                if i == n_items - 1:
                    ones1 = ones_row_h if self.wdt == F16 else ones_row_r
                    brow = st.pop("brow")
                    for j in range(2):
                        nc.tensor.matmul(pss[j][:], ones1[:],
                                         brow[:, j * 512:(j + 1) * 512],
                                         start=False, stop=True)
                    epi(og, pss)

            for i in range(n_items):
                yield (lambda i=i: dma_fn(i)), (lambda i=i: mm_fn(i))

        def lin1_epi(self, og, pss):
            if self.x1 is None:
                self.x1 = geu_sb.tile([NS, self.D], F32, name=f"{self.name}_x1",
                                      tag="geu_x1")
            for j in range(2):
                ot = og * 2 + j
                nc.scalar.copy(self.x1[:, ot * 512:(ot + 1) * 512], pss[j][:])

        def gate_epi(self, og, pss):
            for j in range(2):
                ot = og * 2 + j
                sg = ystream.tile([NS, 512], F32, tag="geu_tmp", name="sg")
                nc.scalar.activation(sg[:], pss[j][:], AF.Sigmoid)
                nc.vector.tensor_tensor(self.x2[:, ot * 512:(ot + 1) * 512],
                                        self.x1[:, ot * 512:(ot + 1) * 512],
                                        sg[:], ALU.mult)
                sq = ystream.tile([NS, 512], F32, tag="geu_tmp", name="sq")
                nc.scalar.activation(sq[:], self.x2[:, ot * 512:(ot + 1) * 512],
                                     AF.Square, accum_out=self.ssb[:, ot:ot + 1])

        def mid(self):
            if debug and self.name == "guv":
                nc.sync.dma_start(d["dbg_gx1"][:, :], self.x1[:, :])
            self.x1T = geu_sb.tile([128, (self.D // 128) * NS], self.wdt,
                                   name=f"{self.name}_x1T", tag="geu_x1T")
            transpose_to_chunks(self.x1, self.D, self.x1T, self.wdt)
            self.x2 = geu_sb.tile([NS, self.D], F32, name=f"{self.name}_x2",
                                  tag="geu_x2")
            self.ssb = small.tile([NS, 8], F32, name=f"{self.name}_ssb", tag="ssb")

        def norm(self):
            if self.D // 512 < 8:
                nc.vector.memset(self.ssb[:, self.D // 512: 8], 0.0)
            ss = small.tile([NS, 1], F32, name=f"{self.name}_ss")
            nc.vector.reduce_sum(ss[:], self.ssb[:], axis=mybir.AxisListType.X)
            ssm = small.tile([NS, 1], F32, name=f"{self.name}_ssm")
            nc.vector.tensor_scalar_max(ssm[:], ss[:], 1e-24)
            sq_ = small.tile([NS, 1], F32, name=f"{self.name}_sq")
            nc.scalar.activation(sq_[:], ssm[:], AF.Sqrt)
            self.inv = small.tile([NS, 1], F32, name=f"{self.name}_inv")
            nc.vector.reciprocal(self.inv[:], sq_[:])

        def fin(self):
            if debug and self.name == "guv":
                nc.sync.dma_start(d["dbg_gx2"][:, :], self.x2[:, :])
            self.norm()
            for ot in range(self.D // 512):
                o_sb = ystream.tile([NS, 512], F32, tag="geu_tmp", name="o_sb")
                nc.vector.tensor_scalar_mul(o_sb[:], self.x2[:, ot * 512:(ot + 1) * 512],
                                            self.inv[:, 0:1])
                nc.sync.dma_start(d["out"][self.out_row, :, ot * 512:(ot + 1) * 512],
                                  o_sb[:])

    # ---------------- conv1: (40 -> 128), k=1 over time, relu ----------------
    w1 = consts.tile([40, 128], F32R)
    nc.sync.dma_start(w1[:], d["w1T"][:, :])
    for s in range(NS):
        for t0 in range(0, 2048, 512):
            ain = ystream.tile([40, 512], F32R, tag="ain", bufs=3)
            nc.sync.dma_start(ain[:], d["aT"][:, s * 2048 + t0: s * 2048 + t0 + 512])
            ps = psum_conv.tile([128, 512], F32, tag="cps")
            nc.tensor.matmul(ps[:], w1[:], ain[:], start=True, stop=True)
            nc.scalar.activation(X2[:, s * 2058 + 5 + t0: s * 2058 + 5 + t0 + 512],
                                 ps[:], AF.Relu, bias=b1t[:, 0:1])

    if debug:
        nc.sync.dma_start(d["dbg_x2"][:, :], X2[:, :].bitcast(F32))

    def maxpool_into(dst_ap, ybuf, width):
        """dst[j] = max(y[2j], y[2j+1], y[2j+2]) over padded ybuf [128, 2*width+2]."""
        even = ybuf[:, 0:2 * width].rearrange("p (j two) -> p j two", two=2)
        odd2 = ybuf[:, 2:2 * width + 2].rearrange("p (j two) -> p j two", two=2)
        m1 = ystream.tile([128, width], F32, tag="ybuf")
        nc.vector.tensor_tensor(m1[:], even[:, :, 0], even[:, :, 1], ALU.max)
        nc.vector.tensor_tensor(dst_ap, m1[:], odd2[:, :, 0], ALU.max)

    # ---------------- text branch: SentenceMaxpool -> tT16 chunks ------------
    tT16 = geu_sb.tile([128, 32 * NS], F16)
    kszs = [128, 128, 44]
    tTin = []
    for ki, kp in enumerate(kszs):
        t_ = consts.tile([kp, NS * 30], F32R, name=f"tTin{ki}")
        nc.sync.dma_start(t_[:], d["tT"][ki * 128: ki * 128 + kp, :])
        tTin.append(t_)
    for o in range(32):
        ps = psum_conv.tile([128, NS * 30], F32, tag="tps", bufs=1)
        for ki, kp in enumerate(kszs):
            wt = wstream.tile([128, 128], F32R, tag="tpw", bufs=4)
            nc.sync.dma_start(wt[0:kp, :], d["tpT"][ki * 128: ki * 128 + kp,
                                                    o * 128:(o + 1) * 128])
            nc.tensor.matmul(ps[:], wt[0:kp, :], tTin[ki][:],
                             start=(ki == 0), stop=(ki == 2))
        tw = ystream.tile([128, NS * 30], F32, tag="tw")
        nc.scalar.activation(tw[:], ps[:], AF.Relu, bias=tpbt[:, o:o + 1])
        tmax = ystream.tile([128, NS], F32, tag="tmax")
        nc.vector.reduce_max(tmax[:], tw[:].rearrange("p (s w) -> p s w", s=NS),
                             axis=mybir.AxisListType.X, opt_input=False)
        nc.vector.tensor_copy(tT16[:, o * NS:(o + 1) * NS], tmax[:])

    if debug:
        nc.gpsimd.dma_start(d["dbg_t"][:, :], tT16[:, :])

    # ---------------- video branch: clip max + norm -> vT16 chunks -----------
    vT16 = geu_sb.tile([128, 32 * NS], F16)
    vchbuf = geu_sb.tile([128, 32 * NS], F32)
    ssv_ps = psum_ss.tile([1, NS], F32)
    for c in range(32):
        vin = ystream.tile([128, NS * 16], F32R, tag="vin")
        nc.sync.dma_start(vin[:], d["vT"][c * 128:(c + 1) * 128, :])
        nc.vector.reduce_max(vchbuf[:, c * NS:(c + 1) * NS],
                             vin[:].rearrange("p (s k) -> p s k", s=NS),
                             axis=mybir.AxisListType.X, opt_input=False)
        vsq = ystream.tile([128, NS], F32R, tag="vsq")
        nc.vector.tensor_tensor(vsq[:], vchbuf[:, c * NS:(c + 1) * NS],
                                vchbuf[:, c * NS:(c + 1) * NS], ALU.mult)
        nc.tensor.matmul(ssv_ps[:], ones_r[:], vsq[:], start=(c == 0), stop=(c == 31))
    ssv = small.tile([1, NS], F32)
    nc.vector.tensor_scalar_max(ssv[:], ssv_ps[:], 1e-24)
    ssq = small.tile([1, NS], F32)
    nc.scalar.activation(ssq[:], ssv[:], AF.Sqrt)
    ssr = small.tile([1, NS], F32)
    nc.vector.reciprocal(ssr[:], ssq[:])
    invb = consts.tile([128, NS], F32)
    nc.gpsimd.partition_broadcast(invb[:], ssr[:])
    for c in range(32):
        nc.vector.tensor_tensor(vT16[:, c * NS:(c + 1) * NS],
                                vchbuf[:, c * NS:(c + 1) * NS], invb[:], ALU.mult)

    if debug:
        nc.gpsimd.dma_start(d["dbg_v"][:, :], vT16[:, :])

    # ---------------- interleave scheduler: GEU weight streams under convs ---
    gut = GeuPlan("gut", lambda: tT16, 32, 4096, "gutf", "gutc", F16, out_row=0)
    guv = GeuPlan("guv", lambda: vT16, 32, 4096, "guvf", "guvc", F16, out_row=1)
    from collections import deque
    _items = deque()
    _items.extend(gut.items())
    _items.extend(guv.items())
    _pending = deque()

    def step():
        if _items:
            dma_fn, mm_fn = _items.popleft()
            if dma_fn is not None:
                dma_fn()
            _pending.append(mm_fn)
            if len(_pending) > 1:
                _pending.popleft()()
        elif _pending:
            _pending.popleft()()

    def flush():
        while _items or _pending:
            step()

    # ---------------- conv2: 128 -> 256, k=11, relu, pool ----------------
    for s in range(NS):
        for co in range(2):
            wt = wstream.tile([128, 11 * 128], F32R, tag="wc")
            nc.sync.dma_start(wt[:], d["w2"][co, :, :])
            yb = ystream.tile([128, 2050], F32, tag="ybuf")
            nc.vector.memset(yb[:, 0:1], NEG)
            nc.vector.memset(yb[:, 2049:2050], NEG)
            for t0 in range(0, 2048, 512):
                ps = psum_conv.tile([128, 512], F32, tag="cps")
                for tap in range(11):
                    nc.tensor.matmul(ps[:], wt[:, tap * 128:(tap + 1) * 128],
                                     X2[:, s * 2058 + t0 + tap: s * 2058 + t0 + tap + 512],
                                     start=(tap == 0), stop=(tap == 10))
                nc.scalar.activation(yb[:, 1 + t0: 1 + t0 + 512], ps[:], AF.Relu,
                                     bias=b2t[:, co:co + 1])
                step()
            maxpool_into(x3c(co)[:, s * 1040 + 8: s * 1040 + 8 + 1024], yb, 1024)

    if debug:
        for c in range(2):
            nc.sync.dma_start(d["dbg_x3"][c * 128:(c + 1) * 128, :], x3c(c).bitcast(F32))

    # ---------------- conv3: 256 -> 512, k=17, relu, pool ----------------
    X4 = acts.tile([128, 4 * NS * 528], F32R, tag="slot1")

    def x4c(c):
        return X4[:, c * (NS * 528):(c + 1) * (NS * 528)]

    zero_halos(X4[:, :], 4, 528, 8)

    for s in range(NS):
        for co in range(4):
            yb = ystream.tile([128, 1026], F32, tag="ybuf")
            nc.vector.memset(yb[:, 0:1], NEG)
            nc.vector.memset(yb[:, 1025:1026], NEG)
            pss = []
            for t0 in range(0, 1024, 512):
                pss.append(psum_conv.tile([128, 512], F32, tag="cps", name="cps"))
            for ci in range(2):
                wt = wstream.tile([128, 17 * 128], F32R, tag="wc")
                nc.sync.dma_start(wt[:], d["w3"][co * 2 + ci, :, :])
                for ti, t0 in enumerate((0, 512)):
                    for tap in range(17):
                        nc.tensor.matmul(pss[ti][:], wt[:, tap * 128:(tap + 1) * 128],
                                         x3c(ci)[:, s * 1040 + t0 + tap: s * 1040 + t0 + tap + 512],
                                         start=(ci == 0 and tap == 0),
                                         stop=(ci == 1 and tap == 16))
            for ti, t0 in enumerate((0, 512)):
                nc.scalar.activation(yb[:, 1 + t0: 1 + t0 + 512], pss[ti][:], AF.Relu,
                                     bias=b3t[:, co:co + 1])
                step()
            maxpool_into(x4c(co)[:, s * 528 + 8: s * 528 + 8 + 512], yb, 512)

    # ---------------- conv4: 512 -> 512, k=17, relu, pool ----------------
    X5 = acts.tile([128, 4 * NS * 272], F32R, tag="slot2")

    def x5c(c):
        return X5[:, c * (NS * 272):(c + 1) * (NS * 272)]

    zero_halos(X5[:, :], 4, 272, 8)

    for s in range(NS):
        for co in range(4):
            yb = ystream.tile([128, 514], F32, tag="ybuf")
            nc.vector.memset(yb[:, 0:1], NEG)
            nc.vector.memset(yb[:, 513:514], NEG)
            ps = psum_conv.tile([128, 512], F32, tag="cps")
            for ci in range(4):
                wt = wstream.tile([128, 17 * 128], F32R, tag="wc")
                nc.sync.dma_start(wt[:], d["w4"][co * 4 + ci, :, :])
                for tap in range(17):
                    nc.tensor.matmul(ps[:], wt[:, tap * 128:(tap + 1) * 128],
                                     x4c(ci)[:, s * 528 + tap: s * 528 + tap + 512],
                                     start=(ci == 0 and tap == 0),
                                     stop=(ci == 3 and tap == 16))
            nc.scalar.activation(yb[:, 1:513], ps[:], AF.Relu, bias=b4t[:, co:co + 1])
            step()
            step()
            maxpool_into(x5c(co)[:, s * 272 + 8: s * 272 + 8 + 256], yb, 256)

    if debug:
        for c in range(4):
            nc.sync.dma_start(d["dbg_x5"][c * 128:(c + 1) * 128, :], x5c(c).bitcast(F32))

    # ---------------- conv5: 512 -> 1024, k=17, relu, pool (paired samples) ----
    A = acts.tile([128, 8 * NS * 128], F32R, tag="slot1")

    def ac(c):  # conv5 output channel chunk c: [128, NS*128]
        return A[:, c * (NS * 128):(c + 1) * (NS * 128)]

    for co in range(8):
        for pr in range(NS // 2):
            s0 = 2 * pr
            yb = ystream.tile([128, 2 * 258], F32, tag="ybuf")
            ybv = yb[:, :].rearrange("p (h x) -> p h x", h=2)
            nc.vector.memset(ybv[:, :, 0:1], NEG)
            nc.vector.memset(ybv[:, :, 257:258], NEG)
            ps = psum_conv.tile([128, 512], F32, tag="cps")
            for ci in range(4):
                wt = wstream.tile([128, 17 * 128], F32R, tag="wc")
                nc.sync.dma_start(wt[:], d["w5"][co * 4 + ci, :, :])
                x5v = x5c(ci).rearrange("p (s x) -> p s x", s=NS)
                for tap in range(17):
                    nc.tensor.matmul(ps[:], wt[:, tap * 128:(tap + 1) * 128],
                                     x5v[:, s0:s0 + 2, tap:tap + 256],
                                     start=(ci == 0 and tap == 0),
                                     stop=(ci == 3 and tap == 16))
            step()
            step()
            nc.scalar.activation(ybv[:, :, 1:257],
                                 ps[:].rearrange("p (h x) -> p h x", h=2),
                                 AF.Relu, bias=b5t[:, co:co + 1])
            for h in range(2):
                maxpool_into(ac(co)[:, (s0 + h) * 128:(s0 + h + 1) * 128],
                             yb[:, h * 258:(h + 1) * 258], 128)

    if debug:
        for c in range(8):
            nc.sync.dma_start(d["dbg_a"][c * 128:(c + 1) * 128, :], ac(c).bitcast(F32))

    flush()

    # ---------------- masked mean over time -> xTg [128, NS] per chunk -------
    xTg = geu_sb.tile([128, 8 * NS], F32R)  # gua lhsT chunks
    for c in range(8):
        for s in range(NS):
            scr = ystream.tile([128, 128], F32, tag="ybuf")
            nc.vector.scalar_tensor_tensor(
                scr[:], ac(c)[:, s * 128:(s + 1) * 128], 1.0, mbs[s][:],
                ALU.mult, ALU.mult, accum_out=xTg[:, c * NS + s: c * NS + s + 1])

    if debug:
        nc.sync.dma_start(d["dbg_pool"][:, :], xTg[:, :].bitcast(F32))

    # audio GEU + projection -> out[2] (runs after masked mean)
    gua = GeuPlan("gua", lambda: xTg, 8, 1024, "guaf", "guac", F32R)
    for dma_fn, mm_fn in gua.items():
        if dma_fn is not None:
            dma_fn()
        mm_fn()
    gua.norm()
    x2an = geu_sb.tile([NS, 1024], F32, name="gua_x2n")
    nc.vector.tensor_scalar_mul(x2an[:], gua.x2[:], gua.inv[:, 0:1])
    a_gT = geu_sb.tile([128, 8 * NS], F32R)
    transpose_to_chunks(x2an, 1024, a_gT, F32R)
    for og in range(4):
        pss = [psum_geu.tile([NS, 512], F32, tag=f"gps{j}", name=f"gps{j}")
               for j in range(2)]
        brow = small.tile([1, 1024], F32R, tag="brow", name="brow")
        nc.sync.dma_start(brow[:], d["projb"][0:1, og * 1024:(og + 1) * 1024])
        for i in range(4):
            wt = gstream.tile([128, 2048], F32R, tag="gw", name="gw")
            nc.sync.dma_start(wt[:], d["projT"][og, :, i * 2048:(i + 1) * 2048])
            for k in range(2):
                kk = i * 2 + k
                for j in range(2):
                    nc.tensor.matmul(pss[j][:], a_gT[:, kk * NS:(kk + 1) * NS],
                                     wt[:, k * 1024 + j * 512: k * 1024 + (j + 1) * 512],
                                     start=(kk == 0), stop=False)
        for j in range(2):
            nc.tensor.matmul(pss[j][:], ones_row_r[:], brow[:, j * 512:(j + 1) * 512],
                             start=False, stop=True)
        for j in range(2):
            ot = og * 2 + j
            ot_sb = ystream.tile([NS, 512], F32, tag="geu_tmp", name="ot_sb")
            nc.scalar.copy(ot_sb[:], pss[j][:])
            nc.sync.dma_start(d["out"][2, :, ot * 512:(ot + 1) * 512], ot_sb[:])


def build(debug=False):
    nc = bacc.Bacc()
    d = declare_io(nc, debug=debug)
    with tile.TileContext(nc) as tc:
        with ExitStack() as ctx:
            emit(ctx, tc, d, debug=debug)
    nc.compile()
    return nc


# ---------------------------------------------------------------------------
# host-side data prep
# ---------------------------------------------------------------------------
def prep_weights(inp):
    """Returns dict of replicated (shared) weight arrays, host-transposed."""
    f32 = np.float32
    w = {}
    bn_scale = (inp["bn_g"][0] / np.sqrt(np.float32(1.0) + np.float32(1e-5))).astype(f32)
    c1 = np.asarray(inp["c1w"])[:, 0, :, 0].astype(f32)   # (128, 40)
    w["w1T"] = np.ascontiguousarray((c1 * bn_scale).T)    # (40, 128)
    w["b1"] = np.ascontiguousarray(
        (np.asarray(inp["c1b"]) + np.asarray(inp["bn_b"])[0] * c1.sum(1)).astype(f32)[:, None])

    def conv_w(cw, coutp, cinp, taps):
        # cw (Cout, Cin, 1, taps) -> (coutp*cinp, cin128, taps*cout128)
        cw = np.asarray(cw)
        ci = cw.shape[1]
        cin = ci // cinp
        a = cw[:, :, 0, :].astype(f32)                    # (Cout, Cin, taps)
        a = a.reshape(coutp, 128, cinp, cin, taps)
        a = a.transpose(0, 2, 3, 4, 1)                    # coutp, cinp, cin, tap, cout
        return np.ascontiguousarray(a.reshape(coutp * cinp, cin, taps * 128))

    def bias_t(b, coutp):
        return np.ascontiguousarray(np.asarray(b).astype(f32).reshape(coutp, 128).T)

    w["w2"] = conv_w(inp["c2w"], 2, 1, 11); w["b2"] = bias_t(inp["c2b"], 2)
    w["w3"] = conv_w(inp["c3w"], 4, 2, 17); w["b3"] = bias_t(inp["c3b"], 4)
    w["w4"] = conv_w(inp["c4w"], 4, 4, 17); w["b4"] = bias_t(inp["c4b"], 4)
    w["w5"] = conv_w(inp["c5w"], 8, 4, 17); w["b5"] = bias_t(inp["c5b"], 8)

    w["tpT"] = np.ascontiguousarray(np.asarray(inp["tp_w"]).astype(f32).T)  # (300, 4096)
    w["tpb"] = np.ascontiguousarray(np.asarray(inp["tp_b"]).astype(f32).reshape(32, 128).T)

    def geu_pack(wm, dt_, og_w=1024):
        # w (O, I) -> wT (I, O) -> (n_og, 128, (I/128)*og_w), k-major per og
        wT = np.asarray(wm).astype(f32).T.astype(dt_)
        I, O = wT.shape
        nk, n_og = I // 128, O // og_w
        a = wT.reshape(nk, 128, n_og, og_w).transpose(2, 1, 0, 3)
        return np.ascontiguousarray(a.reshape(n_og, 128, nk * og_w))

    for nm, src in (("gutf", "gut_fw"), ("gutc", "gut_cw"),
                    ("guvf", "guv_fw"), ("guvc", "guv_cw")):
        w[nm + "T"] = geu_pack(inp[src], np.float16)
        w[nm + "b"] = np.ascontiguousarray(
            np.asarray(inp[src.replace("w", "b")]).astype(np.float16)[None, :])
    for nm, src in (("guaf", "gua_fw"), ("guac", "gua_cw")):
        w[nm + "T"] = geu_pack(inp[src], f32)
        w[nm + "b"] = np.ascontiguousarray(
            np.asarray(inp[src.replace("w", "b")]).astype(f32)[None, :])
    w["projT"] = geu_pack(inp["proj_w"], f32)
    w["projb"] = np.ascontiguousarray(np.asarray(inp["proj_b"]).astype(f32)[None, :])
    return w


def prep_core_inputs(inp, w, b0):
    """Per-core input map for batch slice [b0, b0+NS)."""
    f32 = np.float32
    m = dict(w)
    sl = slice(b0, b0 + NS)
    m["aT"] = np.ascontiguousarray(
        np.asarray(inp["audio"])[sl].astype(f32).transpose(1, 0, 2).reshape(40, NS * 2048))
    m["tT"] = np.ascontiguousarray(
        np.asarray(inp["text"])[sl].astype(f32).transpose(2, 0, 1).reshape(300, NS * 30))
    m["vT"] = np.ascontiguousarray(
        np.asarray(inp["video"])[sl].astype(f32).transpose(2, 0, 1).reshape(4096, NS * 16))
    m["nf"] = np.ascontiguousarray(
        np.asarray(inp["audio_STFT_nframes"])[sl].astype(np.int32)[:, None])
    return m


# ---------------------------------------------------------------------------
# public entry point
# ---------------------------------------------------------------------------
_NC_CACHE = {}


def _get_nc(debug=False):
    if debug not in _NC_CACHE:
        _NC_CACHE[debug] = build(debug=debug)
    return _NC_CACHE[debug]


def kernel(**inputs):
    from concourse.bass_utils import run_bass_kernel_spmd

    nc = _get_nc()
    w = prep_weights(inputs)
    n_cores = 8
    in_maps = [prep_core_inputs(inputs, w, core * NS) for core in range(n_cores)]
    res = run_bass_kernel_spmd(nc, in_maps, core_ids=list(range(n_cores)))
    out = np.empty((3, n_cores * NS, 4096), np.float32)
    for core in range(n_cores):
        out[:, core * NS:(core + 1) * NS, :] = res.results[core]["out"]
    return out

